# revision 3
# baseline (speedup 1.0000x reference)
"""BitNet ternary linear (nn_BitNetLinear4Bit) Trainium2 Bass kernel.

out = x @ (alpha * clip(round(w/alpha), -1, 1))^T + bias
  x: [2, 2048, 4096] f32, w: [11008, 4096] f32, alpha: [1] f32, bias: [11008] f32
  -> out: [2, 2048, 11008] f32

Sharding: column-parallel over 8 cores. Each core gets the full x
(replicated) and a 1376-row slice of w / bias; it produces a
[4096, 1376] slice of the output which the host concatenates.

Per-core algorithm (all math on device):
  Phase W: stream w-shard 128-row x 1024-col blocks, ternarize
    (t = (w >= a/2) - (w <= -a/2), exact in bf16; equals
    clip(round(w/a),-1,1) away from the measure-zero half-integer
    boundary; compare ops alternate DVE/GpSimd per block to halve the
    critical path), then DMA-XBAR-transpose (not PE!) into resident
    transposed weights. K is split: levels ko<16 (k<2048) are kept as
    fp8e4 pairs for DoubleRow matmuls (ternary values are exact in
    fp8), levels ko>=16 stay bf16.
  Phase MM: per 128-token block: SWDGE casting DMA loads x rows
    directly as bf16, DMA-XBAR transpose to xt [128, 32, 128]; the
    first 16 k-levels are DVE-cast to fp8 xt8 [128, 16, 128]. Per
    output group: 8 DoubleRow fp8 matmuls (256-deep contraction each)
    + 16 bf16 matmuls accumulate K=4096 into one PSUM bank; evict
    with a single DVE scalar_tensor_tensor (psum*alpha + bias); DMA
    out. fp8 on half of K gives rel err ~1.7e-2 (< 2e-2 tolerance,
    sim-verified on the fixed-seed inputs).

alpha is read on the host and baked into the program as an immediate;
the compiled program is cached keyed on alpha and recompiled if it
changes.
"""

import numpy as np

B, S, DIN, DOUT = 2, 2048, 4096, 11008
NCORES = 8
DOUT_SH = DOUT // NCORES  # 1376
TOK = B * S  # 4096
P = 128

KF8 = 2048  # k range [0, KF8) computed in fp8 DoubleRow; must be mult of 256
KOF = KF8 // P  # 16 fp8 ko levels
KPAIR = KOF // 2  # 8 DoubleRow matmuls per group per token block
KOB = DIN // P - KOF  # 16 bf16 ko levels


def _build(alpha_f, TOK=TOK, DIN=DIN, DOUT_SH=DOUT_SH, debug=False):
    import concourse.mybir as mybir
    from concourse import bacc
    from concourse.tile import TileContext

    f32 = mybir.dt.float32
    bf16 = mybir.dt.bfloat16
    f8 = mybir.dt.float8e4
    Alu = mybir.AluOpType
    DR = mybir.MatmulPerfMode.DoubleRow

    KO = DIN // P  # 32
    M_SUBS = TOK // P  # 32
    W_CHUNKS = (DOUT_SH + P - 1) // P  # 11 (last chunk 96 rows, zero-padded)
    QCOL = 1024
    QK = QCOL // P  # 8 ko levels per quantize block
    QF8 = KF8 // QCOL  # first QF8 quantize blocks are the fp8 part
    # output groups: one psum bank each, up to 4 chunks (<=512 cols incl pad)
    GROUPS = []  # (first chunk, n chunks, dout start, real width)
    c = 0
    while c < W_CHUNKS:
        cc = min(4, W_CHUNKS - c)
        width = min(DOUT_SH, (c + cc) * P) - c * P  # real (unpadded) width
        GROUPS.append((c, cc, c * P, width))
        c += cc

    a2 = float(alpha_f) * 0.5

    nc = bacc.Bacc(None, target_bir_lowering=False, debug=debug)
    x_d = nc.dram_tensor("x", [TOK, DIN], f32, kind="ExternalInput")
    w_d = nc.dram_tensor("w", [DOUT_SH, DIN], f32, kind="ExternalInput")
    nc.dram_tensor("alpha", [1], f32, kind="ExternalInput")
    b_d = nc.dram_tensor("bias", [DOUT_SH], f32, kind="ExternalInput")
    o_d = nc.dram_tensor("out", [TOK, DOUT_SH], f32, kind="ExternalOutput")

    with TileContext(nc) as tc:
        with (
            tc.tile_pool(name="const", bufs=1) as const,
            tc.tile_pool(name="wres", bufs=1) as wres,
        ):
            bias_sb = const.tile([P, DOUT_SH], f32)
            nc.sync.dma_start(
                bias_sb[:],
                b_d[:].rearrange("(a n) -> a n", a=1).to_broadcast((P, DOUT_SH)),
            )

            # resident transposed ternary weights, chunk-major free layout:
            # bf16 part: wtb[g][p, i, kb, j] = t[(c0+i)*128 + j, (KOF+kb)*128 + p]
            # fp8 part:  wtf[g][p, ko, i*128 + j] = t[(c0+i)*128 + j, ko*128 + p]
            wtb = [
                wres.tile([P, cc, KOB, P], bf16, name=f"wtb_{g}")
                for g, (_, cc, _, _) in enumerate(GROUPS)
            ]
            wtf = [
                wres.tile([P, KOF, cc * P], f8, name=f"wtf_{g}")
                for g, (_, cc, _, _) in enumerate(GROUPS)
            ]

            # ---- Phase W: quantize + DMA-XBAR transpose w shard ----
            with tc.tile_pool(name="wq", bufs=4) as wq:
                for g, (c0, cc, n0, width) in enumerate(GROUPS):
                    for i in range(cc):
                        c = c0 + i
                        rc = min(P, DOUT_SH - c * P)  # 128 or 96 (last)
                        for q in range(DIN // QCOL):
                            wrow = wq.tile([P, QCOL], f32, tag="wrow")
                            if rc < P:
                                nc.gpsimd.memset(wrow[:], 0.0)
                            nc.sync.dma_start(
                                wrow[:rc, :],
                                w_d[c * P : c * P + rc, q * QCOL : (q + 1) * QCOL],
                            )
                            # t = (w >= a/2) - (w <= -a/2) in {-1,0,1}
                            # compares on DVE (exact, f32 in / bf16 out);
                            # subtract on GpSimd to shorten the DVE chain
                            le = wq.tile([P, QCOL], bf16, tag="le")
                            nc.vector.tensor_scalar(
                                le[:], wrow[:], -a2, None, Alu.is_le
                            )
                            ge = wq.tile([P, QCOL], bf16, tag="ge")
                            nc.vector.tensor_scalar(
                                ge[:], wrow[:], a2, None, Alu.is_ge
                            )
                            tq = wq.tile([P, QCOL], bf16, tag="tq")
                            nc.gpsimd.tensor_tensor(
                                tq[:], ge[:], le[:], Alu.subtract
                            )
                            if q < QF8:
                                # fp8 part: XBAR-transpose to a temp, then
                                # DVE-cast into the resident fp8 tensor
                                wtT = wq.tile([P, QK, P], bf16, tag="wtT")
                                nc.sync.dma_start_transpose(wtT[:], tq[:])
                                nc.vector.tensor_copy(
                                    wtf[g][:, q * QK : (q + 1) * QK,
                                           i * P : (i + 1) * P],
                                    wtT[:],
                                )
                            else:
                                # bf16 part: XBAR-transpose straight into the
                                # resident tensor (contiguous free dest)
                                qb = q - QF8
                                nc.sync.dma_start_transpose(
                                    wtb[g][:, i, qb * QK : (qb + 1) * QK, :],
                                    tq[:],
                                )

            # ---- Phase MM ----
            with (
                tc.tile_pool(name="xp", bufs=2) as xp,
                tc.tile_pool(name="xtp", bufs=3) as xtp,
                tc.tile_pool(name="x8p", bufs=3) as x8p,
                tc.tile_pool(name="op", bufs=4) as op,
                tc.tile_pool(name="pso", bufs=6, space="PSUM") as pso,
            ):
                for ms in range(M_SUBS):
                    # SWDGE casting DMA: f32 HBM -> bf16 SBUF directly
                    xbf = xp.tile([P, DIN], bf16, tag="xbf")
                    for h in range(2):
                        hw = DIN // 2
                        nc.gpsimd.dma_start(
                            xbf[:, h * hw : (h + 1) * hw],
                            x_d[ms * P : (ms + 1) * P, h * hw : (h + 1) * hw],
                        )
                    xt = xtp.tile([P, KO, P], bf16, tag="xt")
                    nc.sync.dma_start_transpose(xt[:], xbf[:])
                    xt8 = x8p.tile([P, KOF, P], f8, tag="xt8")
                    nc.vector.tensor_copy(xt8[:], xt[:, :KOF, :])

                    for g, (c0, cc, n0, width) in enumerate(GROUPS):
                        po = pso.tile([P, 512], f32, tag="po", name=f"po_{ms}_{g}")
                        pw = cc * P  # padded width (>= real width)
                        for kp in range(KPAIR):
                            nc.tensor.matmul(
                                po[:, :pw],
                                xt8[:, 2 * kp : 2 * kp + 2, :],
                                wtf[g][:, 2 * kp : 2 * kp + 2, :],
                                start=(kp == 0),
                                stop=False,
                                perf_mode=DR,
                            )
                        for kb in range(KOB):
                            nc.tensor.matmul(
                                po[:, :pw],
                                xt[:, KOF + kb, :],
                                wtb[g][:, :, kb, :],
                                start=False,
                                stop=(kb == KOB - 1),
                            )
                        # out = psum * alpha + bias in one DVE op
                        osb = op.tile([P, 512], f32, tag="osb", name=f"osb_{ms}_{g}")
                        nc.vector.scalar_tensor_tensor(
                            osb[:, :width],
                            po[:, :width],
                            float(alpha_f),
                            bias_sb[:, n0 : n0 + width],
                            Alu.mult,
                            Alu.add,
                        )
                        nc.sync.dma_start(
                            o_d[ms * P : (ms + 1) * P, n0 : n0 + width],
                            osb[:, :width],
                        )

    nc.compile()
    return nc


_CACHE = {}


def _get_nc(alpha_f):
    key = float(alpha_f)
    if key not in _CACHE:
        _CACHE[key] = _build(key)
    return _CACHE[key]


def kernel(x, w, alpha, bias):
    from concourse.bass_utils import run_bass_kernel_spmd

    alpha2 = np.ascontiguousarray(np.asarray(alpha, dtype=np.float32).reshape(1))
    nc = _get_nc(alpha2[0])
    x2 = np.ascontiguousarray(np.asarray(x, dtype=np.float32).reshape(TOK, DIN))
    in_maps = []
    for c in range(NCORES):
        in_maps.append(
            {
                "x": x2,
                "w": np.ascontiguousarray(w[c * DOUT_SH : (c + 1) * DOUT_SH]),
                "alpha": alpha2,
                "bias": np.ascontiguousarray(bias[c * DOUT_SH : (c + 1) * DOUT_SH]),
            }
        )
    res = run_bass_kernel_spmd(nc, in_maps, core_ids=list(range(NCORES)))
    outs = [res.results[c]["out"] for c in range(NCORES)]
    out = np.concatenate(outs, axis=1).reshape(B, S, DOUT)
    return np.ascontiguousarray(out.astype(np.float32))


# revision 4
# speedup vs baseline: 1.0523x; 1.0523x over previous
"""BitNet ternary linear (nn_BitNetLinear4Bit) Trainium2 Bass kernel.

out = x @ (alpha * clip(round(w/alpha), -1, 1))^T + bias
  x: [2, 2048, 4096] f32, w: [11008, 4096] f32, alpha: [1] f32, bias: [11008] f32
  -> out: [2, 2048, 11008] f32

Sharding: column-parallel over 8 cores. Each core gets the full x
(replicated) and a 1376-row slice of w / bias; it produces a
[4096, 1376] slice of the output which the host concatenates.

Per-core algorithm (all math on device). Every matmul is an fp8e4
DoubleRow matmul (256-deep contraction per instruction, measured
162 ns vs 216 ns for a 128-deep bf16 matmul => 2.67x per k-tile):
  - k in [0, KF8): "pure" pairs - two k-levels per pair, x in e4m3.
    e4m3 quantization of x costs ~2.4e-2 rel err if used for all of K;
    restricted to half of K it contributes sqrt(0.5)*2.4e-2 ~ 1.7e-2,
    inside the 2e-2 tolerance (verified numerically on the fixed-seed
    inputs: 1.675e-2).
  - k in [KF8, 4096): "hi/lo" pairs - one k-level per pair, the two
    DoubleRow slots hold e4m3(x) and e4m3(x - e4m3(x)); their sum is
    6x MORE accurate than bf16. The ternary weight is duplicated in
    both slots (exact in fp8), so w[0]m[0]+w[1]m[1] = t*(hi+lo).

Phase W: stream w-shard 128x1024 blocks, ternarize on DVE (le/ge
compares in f32 - exact - then subtract), DMA-XBAR-transpose (PE not
involved), ACT-copy-cast into resident fp8 weights (duplicated into
both pair slots for the hi/lo range).

Phase MM: per 128-token block: SWDGE casting DMA loads x rows as
bf16 (Pool engine does nothing else), DMA-XBAR transpose, then
ACT casts hi parts and DVE computes lo parts. 24 DoubleRow matmuls
per output group accumulate K=4096 into one PSUM bank; a single DVE
scalar_tensor_tensor evicts (psum*alpha + bias); DMA out.

Emission is interleaved (W group 0, first PREFIX_MS token blocks of
group 0, W group 1, ...) because engine queues are strict FIFO per
engine: emitting all of phase W first would head-block the MM-phase
DVE/ACT work behind the whole W phase (this cost 365 us of PE idle in
an earlier version).

alpha is read on the host and baked into the program as an immediate;
the compiled program is cached keyed on alpha and recompiled if it
changes.
"""

import numpy as np

B, S, DIN, DOUT = 2, 2048, 4096, 11008
NCORES = 8
DOUT_SH = DOUT // NCORES  # 1376
TOK = B * S  # 4096
P = 128

KF8 = 2048  # k range [0, KF8) in pure-fp8 pairs; must be mult of 256
KOF = KF8 // P  # 16 pure fp8 ko levels
KPAIR = KOF // 2  # 8 pure DoubleRow matmuls per group per token block
KOB = DIN // P - KOF  # 16 hi/lo ko levels
PREFIX_MS = 6  # token blocks emitted group-major for W/MM overlap


def _build(alpha_f, TOK=TOK, DIN=DIN, DOUT_SH=DOUT_SH, debug=False):
    import concourse.mybir as mybir
    from concourse import bacc
    from concourse.tile import TileContext

    f32 = mybir.dt.float32
    bf16 = mybir.dt.bfloat16
    f8 = mybir.dt.float8e4
    Alu = mybir.AluOpType
    Act = mybir.ActivationFunctionType
    DR = mybir.MatmulPerfMode.DoubleRow

    KO = DIN // P  # 32
    M_SUBS = TOK // P  # 32
    W_CHUNKS = (DOUT_SH + P - 1) // P  # 11 (last chunk 96 rows, zero-padded)
    QCOL = 1024
    QK = QCOL // P  # 8 ko levels per quantize block
    QF8 = KF8 // QCOL  # first QF8 quantize blocks are the pure-fp8 part
    GROUPS = []  # (first chunk, n chunks, dout start, real width)
    c = 0
    while c < W_CHUNKS:
        cc = min(4, W_CHUNKS - c)
        width = min(DOUT_SH, (c + cc) * P) - c * P
        GROUPS.append((c, cc, c * P, width))
        c += cc

    a2 = float(alpha_f) * 0.5

    nc = bacc.Bacc(None, target_bir_lowering=False, debug=debug)
    x_d = nc.dram_tensor("x", [TOK, DIN], f32, kind="ExternalInput")
    w_d = nc.dram_tensor("w", [DOUT_SH, DIN], f32, kind="ExternalInput")
    nc.dram_tensor("alpha", [1], f32, kind="ExternalInput")
    b_d = nc.dram_tensor("bias", [DOUT_SH], f32, kind="ExternalInput")
    o_d = nc.dram_tensor("out", [TOK, DOUT_SH], f32, kind="ExternalOutput")

    with TileContext(nc) as tc:
        with (
            tc.tile_pool(name="const", bufs=1) as const,
            tc.tile_pool(name="wres", bufs=1) as wres,
            tc.tile_pool(name="wq", bufs=3) as wq,
            tc.tile_pool(name="xp", bufs=3) as xp,
            tc.tile_pool(name="xtp", bufs=3) as xtp,
            tc.tile_pool(name="x8p", bufs=7) as x8p,
            tc.tile_pool(name="op", bufs=4) as op,
            tc.tile_pool(name="pso", bufs=8, space="PSUM") as pso,
        ):
            bias_sb = const.tile([P, DOUT_SH], f32)
            nc.sync.dma_start(
                bias_sb[:],
                b_d[:].rearrange("(a n) -> a n", a=1).to_broadcast((P, DOUT_SH)),
            )

            # resident transposed ternary fp8 weights:
            # pure:  wtf[g][p, ko, i*128+j] = t[(c0+i)*128+j, ko*128+p]
            # hi/lo: wtd[g][p, kb, s, i*128+j] = t[(c0+i)*128+j, (KOF+kb)*128+p]
            #        (duplicated into both pair slots s=0,1)
            wtf = [
                wres.tile([P, KOF, cc * P], f8, name=f"wtf_{g}")
                for g, (_, cc, _, _) in enumerate(GROUPS)
            ]
            wtd = [
                wres.tile([P, KOB, 2, cc * P], f8, name=f"wtd_{g}")
                for g, (_, cc, _, _) in enumerate(GROUPS)
            ]

            def emit_w_group(g):
                c0, cc, n0, width = GROUPS[g]
                for i in range(cc):
                    c = c0 + i
                    rc = min(P, DOUT_SH - c * P)  # 128 or 96 (last)
                    for q in range(DIN // QCOL):
                        wrow = wq.tile([P, QCOL], f32, tag="wrow")
                        if rc < P:
                            nc.gpsimd.memset(wrow[:], 0.0)
                        nc.sync.dma_start(
                            wrow[:rc, :],
                            w_d[c * P : c * P + rc, q * QCOL : (q + 1) * QCOL],
                        )
                        # t = (w >= a/2) - (w <= -a/2) in {-1,0,1} (DVE, f32)
                        le = wq.tile([P, QCOL], bf16, tag="le")
                        nc.vector.tensor_scalar(
                            le[:], wrow[:], -a2, None, Alu.is_le
                        )
                        tq = wq.tile([P, QCOL], bf16, tag="tq")
                        nc.vector.scalar_tensor_tensor(
                            tq[:], wrow[:], a2, le[:], Alu.is_ge, Alu.subtract
                        )
                        # transpose 8 k-levels at once on the DMA XBAR
                        wtT = wq.tile([P, QK, P], bf16, tag="wtT")
                        nc.sync.dma_start_transpose(wtT[:], tq[:])
                        if q < QF8:
                            nc.scalar.activation(
                                wtf[g][:, q * QK : (q + 1) * QK,
                                       i * P : (i + 1) * P],
                                wtT[:],
                                Act.Copy,
                            )
                        else:
                            qb = q - QF8
                            for s in range(2):
                                nc.scalar.activation(
                                    wtd[g][:, qb * QK : (qb + 1) * QK, s,
                                           i * P : (i + 1) * P],
                                    wtT[:],
                                    Act.Copy,
                                )

            def emit_x_load(ms):
                # SWDGE casting DMA: f32 HBM -> bf16 SBUF (Pool engine)
                xbf = xp.tile([P, DIN], bf16, tag="xbf", name=f"xbf_{ms}")
                for h in range(2):
                    hw = DIN // 2
                    nc.gpsimd.dma_start(
                        xbf[:, h * hw : (h + 1) * hw],
                        x_d[ms * P : (ms + 1) * P, h * hw : (h + 1) * hw],
                    )
                xt = xtp.tile([P, KO, P], bf16, tag="xt", name=f"xt_{ms}")
                nc.sync.dma_start_transpose(xt[:], xbf[:])
                # pure part: hi only; hi/lo part: e4m3(x) and e4m3(x - hi)
                xt8 = x8p.tile([P, KOF, P], f8, tag="xt8", name=f"xt8_{ms}")
                nc.vector.tensor_copy(xt8[:], xt[:, :KOF, :])
                xhl = x8p.tile([P, KOB, 2, P], f8, tag="xhl", name=f"xhl_{ms}")
                nc.scalar.activation(xhl[:, :, 0, :], xt[:, KOF:, :], Act.Copy)
                nc.vector.tensor_tensor(
                    xhl[:, :, 1, :], xt[:, KOF:, :], xhl[:, :, 0, :],
                    Alu.subtract,
                )
                return xt8, xhl

            def emit_mm(ms, g, xt8, xhl):
                c0, cc, n0, width = GROUPS[g]
                po = pso.tile([P, 512], f32, tag="po", name=f"po_{ms}_{g}")
                pw = cc * P
                for kp in range(KPAIR):
                    nc.tensor.matmul(
                        po[:, :pw],
                        xt8[:, 2 * kp : 2 * kp + 2, :],
                        wtf[g][:, 2 * kp : 2 * kp + 2, :],
                        start=(kp == 0),
                        stop=False,
                        perf_mode=DR,
                    )
                for kb in range(KOB):
                    nc.tensor.matmul(
                        po[:, :pw],
                        xhl[:, kb, :, :],
                        wtd[g][:, kb, :, :],
                        start=False,
                        stop=(kb == KOB - 1),
                        perf_mode=DR,
                    )
                osb = op.tile([P, 512], f32, tag="osb", name=f"osb_{ms}_{g}")
                nc.vector.scalar_tensor_tensor(
                    osb[:, :width],
                    po[:, :width],
                    float(alpha_f),
                    bias_sb[:, n0 : n0 + width],
                    Alu.mult,
                    Alu.add,
                )
                nc.sync.dma_start(
                    o_d[ms * P : (ms + 1) * P, n0 : n0 + width],
                    osb[:, :width],
                )

            # interleaved emission: W(g) then the first PREFIX_MS token
            # blocks of group g, so engine FIFOs never head-block the MM
            # pipeline behind the whole W phase.
            x8_pre = {}
            emit_w_group(0)
            for ms in range(PREFIX_MS):
                x8_pre[ms] = emit_x_load(ms)
            for ms in range(PREFIX_MS):
                emit_mm(ms, 0, *x8_pre[ms])
            emit_w_group(1)
            for ms in range(PREFIX_MS):
                emit_mm(ms, 1, *x8_pre[ms])
            emit_w_group(2)
            for ms in range(PREFIX_MS):
                emit_mm(ms, 2, *x8_pre[ms])
            for ms in range(PREFIX_MS, M_SUBS):
                xt8, xhl = emit_x_load(ms)
                for g in range(len(GROUPS)):
                    emit_mm(ms, g, xt8, xhl)

    nc.compile()
    return nc


_CACHE = {}


def _get_nc(alpha_f):
    key = float(alpha_f)
    if key not in _CACHE:
        _CACHE[key] = _build(key)
    return _CACHE[key]


def kernel(x, w, alpha, bias):
    from concourse.bass_utils import run_bass_kernel_spmd

    alpha2 = np.ascontiguousarray(np.asarray(alpha, dtype=np.float32).reshape(1))
    nc = _get_nc(alpha2[0])
    x2 = np.ascontiguousarray(np.asarray(x, dtype=np.float32).reshape(TOK, DIN))
    in_maps = []
    for c in range(NCORES):
        in_maps.append(
            {
                "x": x2,
                "w": np.ascontiguousarray(w[c * DOUT_SH : (c + 1) * DOUT_SH]),
                "alpha": alpha2,
                "bias": np.ascontiguousarray(bias[c * DOUT_SH : (c + 1) * DOUT_SH]),
            }
        )
    res = run_bass_kernel_spmd(nc, in_maps, core_ids=list(range(NCORES)))
    outs = [res.results[c]["out"] for c in range(NCORES)]
    out = np.concatenate(outs, axis=1).reshape(B, S, DOUT)
    return np.ascontiguousarray(out.astype(np.float32))


# revision 7
# speedup vs baseline: 1.2130x; 1.1528x over previous
"""BitNet ternary linear (nn_BitNetLinear4Bit) Trainium2 Bass kernel.

out = x @ (alpha * clip(round(w/alpha), -1, 1))^T + bias
  x: [2, 2048, 4096] f32, w: [11008, 4096] f32, alpha: [1] f32, bias: [11008] f32
  -> out: [2, 2048, 11008] f32

Sharding: column-parallel over 8 cores. Each core gets the full x
(replicated) and a 1376-row slice of w / bias; it produces a
[4096, 1376] slice of the output which the host concatenates.

Per-core algorithm (all math on device). HW measurements that shaped
it (from perfetto traces of earlier versions):
  - matmul issue gap is N/2.4GHz + 2.5ns regardless of dtype or
    perf_mode; an fp8e4 DoubleRow matmul covers TWO 128-deep k-tiles
    per instruction at the same N-cost => 2x throughput per k-tile.
  - e4m3 quantization of x costs 2.35e-2 rel err if applied to all of
    K; applied to half (k < 2048) it contributes sqrt(.5)*2.35e-2 and
    the bf16 other half is exact-ish => total 1.67e-2 < 2e-2 gate
    (verified numerically on the fixed-seed inputs; ternary weights
    are EXACT in fp8/bf16 so they add no error).
  - DMA fixed cost ~2us per transfer: weight-phase transposes are
    batched at [128, 2048] and issued on the *scalar* HWDGE queue
    (sync queue keeps x transposes + w loads; Pool/SWDGE does x
    casting loads + output stores) so no single queue serializes.
  - engine queues are strict FIFO: emission interleaves phase W per
    output group with the first PREFIX_MS token blocks of matmuls so
    the PE starts ~30us in instead of waiting for all of phase W.

Per 128-token block, per output group (512/512/384 cols): 8 DoubleRow
fp8 matmuls (k<2048, x in e4m3) + 16 bf16 matmuls (k>=2048) accumulate
into one PSUM bank; one DVE scalar_tensor_tensor evicts psum*alpha +
bias; SWDGE DMA stores. Ternarize t = (w>=a/2) - (w<=-a/2) runs on DVE
in f32 (exact); transposed via DMA XBAR (PE untouched).

alpha is read on the host and baked into the program as an immediate;
the compiled program is cached keyed on alpha and recompiled if it
changes.
"""

import numpy as np

B, S, DIN, DOUT = 2, 2048, 4096, 11008
NCORES = 8
DOUT_SH = DOUT // NCORES  # 1376
TOK = B * S  # 4096
P = 128

KF8 = 2048  # k range [0, KF8) in pure-fp8 DoubleRow pairs; mult of 256
KOF = KF8 // P  # 16 fp8 ko levels
KPAIR = KOF // 2  # 8 DoubleRow matmuls per group per token block
KOB = DIN // P - KOF  # 16 bf16 ko levels
PREFIX_MS = 4  # token blocks emitted group-major for W/MM overlap


def _build(alpha_f, TOK=TOK, DIN=DIN, DOUT_SH=DOUT_SH, debug=False):
    import concourse.mybir as mybir
    from concourse import bacc
    from concourse.tile import TileContext

    f32 = mybir.dt.float32
    bf16 = mybir.dt.bfloat16
    f8 = mybir.dt.float8e4
    Alu = mybir.AluOpType
    Act = mybir.ActivationFunctionType
    DR = mybir.MatmulPerfMode.DoubleRow

    KO = DIN // P  # 32
    M_SUBS = TOK // P  # 32
    W_CHUNKS = (DOUT_SH + P - 1) // P  # 11 (last chunk 96 rows, zero-padded)
    HCOL = 2048  # w processed in two 2048-col halves: fp8 half, bf16 half
    assert KF8 == HCOL
    GROUPS = []  # (first chunk, n chunks, dout start, real width)
    c = 0
    while c < W_CHUNKS:
        cc = min(4, W_CHUNKS - c)
        width = min(DOUT_SH, (c + cc) * P) - c * P
        GROUPS.append((c, cc, c * P, width))
        c += cc

    a2 = float(alpha_f) * 0.5

    nc = bacc.Bacc(None, target_bir_lowering=False, debug=debug)
    x_d = nc.dram_tensor("x", [TOK, DIN], f32, kind="ExternalInput")
    w_d = nc.dram_tensor("w", [DOUT_SH, DIN], f32, kind="ExternalInput")
    nc.dram_tensor("alpha", [1], f32, kind="ExternalInput")
    b_d = nc.dram_tensor("bias", [DOUT_SH], f32, kind="ExternalInput")
    o_d = nc.dram_tensor("out", [TOK, DOUT_SH], f32, kind="ExternalOutput")

    with TileContext(nc) as tc:
        with (
            tc.tile_pool(name="const", bufs=1) as const,
            tc.tile_pool(name="wres", bufs=1) as wres,
            tc.tile_pool(name="wq", bufs=2) as wq,
            tc.tile_pool(name="xp", bufs=2) as xp,
            tc.tile_pool(name="xtp", bufs=5) as xtp,
            tc.tile_pool(name="x8p", bufs=5) as x8p,
            tc.tile_pool(name="op", bufs=3) as op,
            tc.tile_pool(name="pso", bufs=8, space="PSUM") as pso,
        ):
            bias_sb = const.tile([P, DOUT_SH], f32)
            nc.sync.dma_start(
                bias_sb[:],
                b_d[:].rearrange("(a n) -> a n", a=1).to_broadcast((P, DOUT_SH)),
            )

            # resident transposed ternary weights:
            # fp8:  wtf[g][p, ko, i*128+j] = t[(c0+i)*128+j, ko*128+p]
            # bf16: wtb[g][p, i, kb, j]   = t[(c0+i)*128+j, (KOF+kb)*128+p]
            wtf = [
                wres.tile([P, KOF, cc * P], f8, name=f"wtf_{g}")
                for g, (_, cc, _, _) in enumerate(GROUPS)
            ]
            wtb = [
                wres.tile([P, cc, KOB, P], bf16, name=f"wtb_{g}")
                for g, (_, cc, _, _) in enumerate(GROUPS)
            ]

            def emit_w_group(g):
                c0, cc, n0, width = GROUPS[g]
                for i in range(cc):
                    c = c0 + i
                    rc = min(P, DOUT_SH - c * P)  # 128 or 96 (last)
                    # ternarize the full 4096-col row in two 2048 halves
                    tqf = wq.tile([P, DIN], bf16, tag="tqf")
                    for h in range(2):
                        wrow = wq.tile([P, HCOL], f32, tag="wrow")
                        if rc < P:
                            nc.gpsimd.memset(wrow[:], 0.0)
                        nc.sync.dma_start(
                            wrow[:rc, :],
                            w_d[c * P : c * P + rc, h * HCOL : (h + 1) * HCOL],
                        )
                        # t = (w >= a/2) - (w <= -a/2) in {-1,0,1} (DVE, f32)
                        le = wq.tile([P, HCOL], bf16, tag="le")
                        nc.vector.tensor_scalar(
                            le[:], wrow[:], -a2, None, Alu.is_le
                        )
                        nc.vector.scalar_tensor_tensor(
                            tqf[:, h * HCOL : (h + 1) * HCOL],
                            wrow[:],
                            a2,
                            le[:],
                            Alu.is_ge,
                            Alu.subtract,
                        )
                    # ONE full-row XBAR transpose (sync queue — all
                    # transposes stay on a single queue: concurrent XBAR
                    # use from two HWDGE queues corrupts), then split:
                    # ACT-cast the fp8 half, DVE-copy the bf16 half.
                    wtT = wq.tile([P, KO, P], bf16, tag="wtT")
                    nc.sync.dma_start_transpose(wtT[:], tqf[:])
                    nc.scalar.activation(
                        wtf[g][:, :, i * P : (i + 1) * P],
                        wtT[:, :KOF, :],
                        Act.Copy,
                    )
                    nc.vector.tensor_copy(wtb[g][:, i, :, :], wtT[:, KOF:, :])

            def emit_x_load(ms):
                # SWDGE casting DMA: f32 HBM -> bf16 SBUF (Pool engine)
                xbf = xp.tile([P, DIN], bf16, tag="xbf", name=f"xbf_{ms}")
                for h in range(2):
                    hw = DIN // 2
                    nc.gpsimd.dma_start(
                        xbf[:, h * hw : (h + 1) * hw],
                        x_d[ms * P : (ms + 1) * P, h * hw : (h + 1) * hw],
                    )
                xt = xtp.tile([P, KO, P], bf16, tag="xt", name=f"xt_{ms}")
                nc.sync.dma_start_transpose(xt[:], xbf[:])
                xt8 = x8p.tile([P, KOF, P], f8, tag="xt8", name=f"xt8_{ms}")
                nc.vector.tensor_copy(xt8[:], xt[:, :KOF, :])
                return xt, xt8

            def emit_mm(ms, g, xt, xt8):
                c0, cc, n0, width = GROUPS[g]
                po = pso.tile([P, 512], f32, tag="po", name=f"po_{ms}_{g}")
                pw = cc * P
                for kp in range(KPAIR):
                    nc.tensor.matmul(
                        po[:, :pw],
                        xt8[:, 2 * kp : 2 * kp + 2, :],
                        wtf[g][:, 2 * kp : 2 * kp + 2, :],
                        start=(kp == 0),
                        stop=False,
                        perf_mode=DR,
                    )
                for kb in range(KOB):
                    nc.tensor.matmul(
                        po[:, :pw],
                        xt[:, KOF + kb, :],
                        wtb[g][:, :, kb, :],
                        start=False,
                        stop=(kb == KOB - 1),
                    )
                osb = op.tile([P, 512], f32, tag="osb", name=f"osb_{ms}_{g}")
                nc.vector.scalar_tensor_tensor(
                    osb[:, :width],
                    po[:, :width],
                    float(alpha_f),
                    bias_sb[:, n0 : n0 + width],
                    Alu.mult,
                    Alu.add,
                )
                # store on the SWDGE (Pool) queue to keep sync free
                nc.gpsimd.dma_start(
                    o_d[ms * P : (ms + 1) * P, n0 : n0 + width],
                    osb[:, :width],
                )

            # interleaved emission: W(g) then the first PREFIX_MS token
            # blocks of group g, so strict-FIFO engine queues never
            # head-block the MM pipeline behind the whole W phase.
            x_pre = {}
            emit_w_group(0)
            for ms in range(PREFIX_MS):
                x_pre[ms] = emit_x_load(ms)
            for ms in range(PREFIX_MS):
                emit_mm(ms, 0, *x_pre[ms])
            emit_w_group(1)
            for ms in range(PREFIX_MS):
                emit_mm(ms, 1, *x_pre[ms])
            emit_w_group(2)
            for ms in range(PREFIX_MS):
                emit_mm(ms, 2, *x_pre[ms])
            for ms in range(PREFIX_MS, M_SUBS):
                xt, xt8 = emit_x_load(ms)
                for g in range(len(GROUPS)):
                    emit_mm(ms, g, xt, xt8)

    nc.compile()
    return nc


_CACHE = {}


def _get_nc(alpha_f):
    key = float(alpha_f)
    if key not in _CACHE:
        _CACHE[key] = _build(key)
    return _CACHE[key]


def kernel(x, w, alpha, bias):
    from concourse.bass_utils import run_bass_kernel_spmd

    alpha2 = np.ascontiguousarray(np.asarray(alpha, dtype=np.float32).reshape(1))
    nc = _get_nc(alpha2[0])
    x2 = np.ascontiguousarray(np.asarray(x, dtype=np.float32).reshape(TOK, DIN))
    in_maps = []
    for c in range(NCORES):
        in_maps.append(
            {
                "x": x2,
                "w": np.ascontiguousarray(w[c * DOUT_SH : (c + 1) * DOUT_SH]),
                "alpha": alpha2,
                "bias": np.ascontiguousarray(bias[c * DOUT_SH : (c + 1) * DOUT_SH]),
            }
        )
    res = run_bass_kernel_spmd(nc, in_maps, core_ids=list(range(NCORES)))
    outs = [res.results[c]["out"] for c in range(NCORES)]
    out = np.concatenate(outs, axis=1).reshape(B, S, DOUT)
    return np.ascontiguousarray(out.astype(np.float32))
